# revision 2
# baseline (speedup 1.0000x reference)
"""Trainium2 Bass kernel for nn_MDCR (multi-dilated conv residual block).

Pipeline per batch image (one NeuronCore per batch element, 8 total):
  stage 1: four depthwise 3x3 dilated convs (rates 1/6/12/18, 128 ch each)
           -> +bias -> BN(eval) -> ReLU
  stage 2: shared 1x1 "mix" over the 4 branch outputs (4->4 per channel)
           -> BN -> ReLU
  stage 3: dense 1x1 conv 512->512 -> BN -> ReLU

v2 mapping (vs baseline: fp32 1x DVE taps, all-fp32 accumulators):
  - Branch/engine split chosen for DVE 2x-mode alignment: rates 6 and 12
    have all tap offsets even (4B-aligned in fp16) so they run on the DVE
    at 2x via 16-bit scalar_tensor_tensor chains into fp16 SBUF
    accumulators; rates 1 and 18 run on the PE as diagonal tap matmuls
    into PSUM (alignment-insensitive).  Per-(branch,strip) table ON_PE
    lets stage-1 work shift between PE and DVE for balance.
  - Stage-1 DVE epilogue: one tensor_scalar (bias-add + max(0)) fp16->bf16
    (single-src op, 4x eligible).  PE epilogue: ACT PSUM->SBUF Relu.
  - Stage 2 split o-wise between PE (diag matmuls) and DVE (scalar ptr
    MAC chains), knob ST2_ON_PE.
  - Stage 3: dense 512x512 bf16 on PE as 4x4 blocks of 128x128 over
    512-px PSUM chunks; ACT epilogue -> fp32 -> DMA out.
  - x cast fp32->16bit on the Pool engine (knob CAST_ENGINE).
  - Software pipeline: stage-1 of strip s is emitted one iteration ahead
    of stage-2/3 of strip s-1; input DMA+cast runs two strips ahead.
"""

import ml_dtypes
import numpy as np

import concourse.bass as bass
import concourse.mybir as mybir
import concourse.tile as tile
from concourse.bass_utils import run_bass_kernel_spmd
from concourse.vector_clock import ScopedClock


def _patched_drain_and_barrier(self, tick_clock, wait_clock):
    """This walrus build rejects sync waits on the Drain opcode (CTRL
    NO_STRUCT encoding). Split the kernel-tail drain's aggregated sem waits
    onto individual sync-engine NoOps, then emit a bare drain."""
    nc = self.nc
    collector = nc.sync.nop(nofuse=True, hint="tail_wait_collector")
    wait_clock.add_sem_waits(
        collector.ins, ScopedClock({None: tick_clock.global_clock}))
    si = collector.ins.sync_info
    waits = list(si.on_wait) if si is not None else []
    if len(waits) > 1:
        collector.ins.sync_info = mybir.SyncInfo(
            on_wait=[waits[0]], on_update=list(si.on_update))
        for w in waits[1:]:
            n = nc.sync.nop(nofuse=True, hint="tail_wait")
            n.ins.sync_info = mybir.SyncInfo(on_wait=[w], on_update=[])
    nc.sync.drain()
    nc.all_engine_barrier()
    assert self.sems is not None
    popped = nc._tile_sem_poison_stack.pop()
    assert popped is self._sem_poison
    nc.clear_and_free_semaphores(list(self.sems.allocated().values()))
    nc.all_engine_barrier()


tile.TileContext._drain_and_barrier = _patched_drain_and_barrier


def _split_multi_waits(nc):
    """This walrus build supports at most one sync-wait per instruction.
    Move extra waits onto same-engine NoOps placed immediately before."""
    for fn in nc.m.functions:
        for blk in fn.blocks:
            insts = blk.instructions
            if not any(i.sync_info and len(i.sync_info.on_wait) > 1
                       for i in insts):
                continue
            out = []
            for ins in insts:
                si = ins.sync_info
                if si is not None and len(si.on_wait) > 1:
                    waits = list(si.on_wait)
                    for w in waits[:-1]:
                        nop = mybir.InstNoOp(
                            name=nc.get_next_instruction_name(),
                            sync_info=mybir.SyncInfo(on_wait=[w], on_update=[]),
                            bass_nofuse=True,
                            engine=ins.engine,
                        )
                        try:
                            nc.register_instruction(nop, overwrite=True)
                        except Exception:
                            pass
                        out.append(nop)
                    ins.sync_info = mybir.SyncInfo(
                        on_wait=[waits[-1]], on_update=list(si.on_update))
                out.append(ins)
            blk.instructions = out


EPS = 1e-5
RATES = (1, 6, 12, 18)
B, C, H, W = 8, 512, 96, 96
CQ = C // 4                # 128, one partition chunk per branch
PIX = H * W
STRIP = 16                 # rows per pipeline strip
SNT = STRIP * W            # 1536 px per strip
N_STRIPS = H // STRIP      # 6
BANK = 512                 # fp32 psum bank columns
RPB = 4                    # rows per psum bank in stage-1 PE path
NT = RPB * W               # 384 used columns per bank
BF16 = mybir.dt.bfloat16
FP16 = mybir.dt.float16
F32 = mybir.dt.float32

# ---- tuning knobs ------------------------------------------------------
# stage-1 engine per (branch, strip): True -> PE diag matmuls, False -> DVE
# branches 1 (r=6) and 2 (r=12) have 4B-aligned tap offsets in fp16 ->
# always DVE; branch 0 (r=1) has odd offsets -> PE; branch 3 (r=18) split.
ON_PE = {}
for _s in range(N_STRIPS):
    ON_PE[(0, _s)] = True
    ON_PE[(1, _s)] = False
    ON_PE[(2, _s)] = False
    ON_PE[(3, _s)] = (_s % 2 == 0)
# stage-2 output o computed on PE (else DVE)
ST2_ON_PE = (True, True, False, False)
# engine for the fp32 -> 16-bit input casts: "pool" | "vector" | "scalar"
CAST_ENGINE = "pool"
# xpad dtype per branch: DVE-only branches fp16 (better mantissa), branches
# touched by PE bf16
XPAD_DT = [BF16, FP16, FP16, BF16]

_PROG_CACHE = {}


def _np_bf16(a):
    return np.asarray(a, dtype=np.float32).astype(ml_dtypes.bfloat16)


def _host_consts(wdw, bdw, gdw, bedw, mdw, vdw, ws, bs, gs, bes, ms, vs,
                 wo, bo, go, beo, mo, vo):
    """Fold BN constants and build PE weight blocks on the host."""
    f64 = np.float64
    # stage 1: y = relu(conv(x; w*s1) + b1)   (BN scale folded into taps)
    inv1 = np.asarray(gdw, f64) / np.sqrt(np.asarray(vdw, f64) + EPS)  # [4,128]
    s1 = inv1
    b1 = (np.asarray(bdw, f64) - np.asarray(mdw, f64)) * inv1 + np.asarray(bedw, f64)

    # stage 2: z_o = relu(sum_i Amix[o,i]*y_i + b2[o])
    invs = np.asarray(gs, f64) / np.sqrt(np.asarray(vs, f64) + EPS)    # [4]
    W4 = np.asarray(ws, f64)[:, :, 0, 0]                               # [o,i]
    Amix = W4 * invs[:, None]
    b2 = (np.asarray(bs, f64) - np.asarray(ms, f64)) * invs + np.asarray(bes, f64)

    # stage 3: out = relu(Wo' z + b3), Wo' = diag(s3) Wo
    inv3 = np.asarray(go, f64) / np.sqrt(np.asarray(vo, f64) + EPS)    # [512]
    Wo = np.asarray(wo, f64)[:, :, 0, 0]                               # [512,512]
    Wo_s = Wo * inv3[:, None]
    b3 = (np.asarray(bo, f64) - np.asarray(mo, f64)) * inv3 + np.asarray(beo, f64)

    # folded per-channel tap weights [4,128,9]
    wtap = np.asarray(wdw, f64)[:, :, 0, :, :].reshape(4, CQ, 9) * s1[:, :, None]
    wtap32 = wtap.astype(np.float32)

    consts = {}
    # depthwise diag blocks: [128(part=k=c), 36(branch*9+tap), 128(m=c)]
    dw = np.zeros((CQ, 4 * 9, CQ), np.float32)
    for i in range(4):
        for t in range(9):
            np.fill_diagonal(dw[:, i * 9 + t, :], wtap32[i, :, t])
    consts["dww"] = _np_bf16(dw)

    # per-channel tap scalars for the DVE path: [128, 36], col = i*9+t
    consts["dws"] = np.ascontiguousarray(
        wtap32.transpose(1, 0, 2).reshape(CQ, 36))

    # mix blocks: [128(k=c), 16(o*4+i), 128(m=c)] = Amix[o,i] * I
    mixw = np.zeros((CQ, 16, CQ), np.float32)
    for o in range(4):
        for i in range(4):
            np.fill_diagonal(mixw[:, o * 4 + i, :], np.float32(Amix[o, i]))
    consts["mixw"] = _np_bf16(mixw)

    # mix scalars broadcast across partitions: [128, 16] col = o*4+i
    consts["amix"] = np.ascontiguousarray(
        np.broadcast_to(Amix.reshape(1, 16).astype(np.float32),
                        (CQ, 16))).copy()

    # stage-3 blocks: [128(k=c of z-chunk o), 16(m*4+o), 128(mc)]
    # z-chunk o, row c  <->  original z channel 4c+o
    s3w = np.zeros((CQ, 16, CQ), np.float32)
    for m in range(4):
        blk = Wo_s[128 * m:128 * (m + 1), :].astype(np.float32)  # [mc, 512]
        for o in range(4):
            s3w[:, m * 4 + o, :] = blk[:, o::4].T  # [c, mc]
    consts["s3w"] = _np_bf16(s3w)

    consts["s1b"] = np.ascontiguousarray(np.asarray(b1, np.float32).T)  # [128,4]
    consts["b2bc"] = np.ascontiguousarray(
        np.broadcast_to(np.asarray(b2, np.float32)[None, :], (CQ, 4))).copy()
    consts["b3"] = np.ascontiguousarray(
        np.asarray(b3, np.float32).reshape(4, CQ).T)  # [128,4] col m
    return consts


def _tap_list(r, h0, h1, order=None):
    """Taps (t, dh, dw, lo, hi) clamped to rows [h0,h1); center first."""
    taps = []
    for t in (order or range(9)):
        dh, dw = t // 3 - 1, t % 3 - 1
        lo = max(h0, -dh * r)
        hi = min(h1, H - dh * r)
        if lo < hi:
            taps.append((t, dh, dw, lo, hi))
    taps.sort(key=lambda e: (e[3] != h0 or e[4] != h1,))
    return taps


def _build_program(loop_n=None):
    nc = bass.Bass("TRN2", target_bir_lowering=False, debug=False, num_devices=8)

    x_d = nc.dram_tensor("x", [C, H, W], F32, kind="ExternalInput")
    dws_d = nc.dram_tensor("dws", [CQ, 36], F32, kind="ExternalInput")
    dww_d = nc.dram_tensor("dww", [CQ, 36, CQ], BF16, kind="ExternalInput")
    mixw_d = nc.dram_tensor("mixw", [CQ, 16, CQ], BF16, kind="ExternalInput")
    amix_d = nc.dram_tensor("amix", [CQ, 16], F32, kind="ExternalInput")
    s3w_d = nc.dram_tensor("s3w", [CQ, 16, CQ], BF16, kind="ExternalInput")
    s1b_d = nc.dram_tensor("s1b", [CQ, 4], F32, kind="ExternalInput")
    b2bc_d = nc.dram_tensor("b2bc", [CQ, 4], F32, kind="ExternalInput")
    b3_d = nc.dram_tensor("b3", [CQ, 4], F32, kind="ExternalInput")
    out_d = nc.dram_tensor("out", [C, PIX], F32, kind="ExternalOutput")

    cast_eng = {"pool": nc.gpsimd, "vector": nc.vector,
                "scalar": nc.scalar}[CAST_ENGINE]

    with tile.TileContext(nc) as tc:
        with (
            tc.tile_pool(name="consts", bufs=1) as cpool,
            tc.tile_pool(name="xpad", bufs=1) as xpool,
            tc.tile_pool(name="stage", bufs=8) as spool,
            tc.tile_pool(name="acc1", bufs=4) as a1pool,
            tc.tile_pool(name="acc2", bufs=3) as a2pool,
            tc.tile_pool(name="ys", bufs=8) as ypool,
            tc.tile_pool(name="zs", bufs=5) as zpool,
            tc.tile_pool(name="outs", bufs=3) as opool,
            tc.tile_pool(name="ps1", bufs=2, space=bass.MemorySpace.PSUM) as ps1,
            tc.tile_pool(name="ps2", bufs=2, space=bass.MemorySpace.PSUM) as ps2,
            tc.tile_pool(name="ps3", bufs=2, space=bass.MemorySpace.PSUM) as ps3,
        ):
          def _body():
            # ---- constants to SBUF
            dww = cpool.tile([CQ, 36, CQ], BF16, tag="dww")
            nc.sync.dma_start(dww[:], dww_d[:])
            mixw = cpool.tile([CQ, 16, CQ], BF16, tag="mixw")
            nc.sync.dma_start(mixw[:], mixw_d[:])
            s3w = cpool.tile([CQ, 16, CQ], BF16, tag="s3w")
            nc.sync.dma_start(s3w[:], s3w_d[:])
            dws = cpool.tile([CQ, 36], F32, tag="dws")
            nc.sync.dma_start(dws[:], dws_d[:])
            amix = cpool.tile([CQ, 16], F32, tag="amix")
            nc.sync.dma_start(amix[:], amix_d[:])
            s1b = cpool.tile([CQ, 4], F32, tag="s1b")
            nc.sync.dma_start(s1b[:], s1b_d[:])
            b2bc = cpool.tile([CQ, 4], F32, tag="b2bc")
            nc.sync.dma_start(b2bc[:], b2bc_d[:])
            b3 = cpool.tile([CQ, 4], F32, tag="b3")
            nc.sync.dma_start(b3[:], b3_d[:])

            # ---- padded x tiles, W padded by r zeros each side
            xpad = []
            for i, r in enumerate(RATES):
                wp = W + 2 * r
                t = xpool.tile([CQ, H, wp], XPAD_DT[i], tag=f"xpad{i}")
                xpad.append(t)
                nc.gpsimd.memset(t[:, :, 0:r], 0.0)
                nc.gpsimd.memset(t[:, :, r + W:wp], 0.0)

            # ---- input load + cast, 8-row chunks, per branch
            def load_strip(s):
                H0 = s * STRIP
                for half in range(2):
                    g0 = H0 + half * 8
                    g1 = g0 + 8
                    for i, r in enumerate(RATES):
                        st = spool.tile([CQ, 8, W], F32, tag="stage")
                        nc.sync.dma_start(
                            st[:], x_d[CQ * i:CQ * (i + 1), g0:g1, :])
                        cast_eng.tensor_copy(
                            xpad[i][:, g0:g1, r:r + W], st[:])

            # ---- stage 1, one strip, one branch
            def st1_dve(i, s):
                r = RATES[i]
                H0 = s * STRIP
                acc = a1pool.tile([CQ, SNT], FP16, tag="acc1")
                for j, (t, dh, dw, lo, hi) in enumerate(
                        _tap_list(r, H0, H0 + STRIP)):
                    xin = xpad[i][:, lo + dh * r:hi + dh * r,
                                  r + dw * r:r + dw * r + W]
                    sc = dws[:, i * 9 + t:i * 9 + t + 1]
                    sub = acc[:, (lo - H0) * W:(hi - H0) * W]
                    if j == 0:
                        nc.vector.tensor_scalar(sub, xin, sc, None,
                                                mybir.AluOpType.mult)
                    else:
                        nc.vector.scalar_tensor_tensor(
                            sub, xin, sc, sub,
                            mybir.AluOpType.mult, mybir.AluOpType.add)
                yb = ypool.tile([CQ, SNT], BF16, tag="y")
                nc.vector.tensor_scalar(
                    yb[:], acc[:], s1b[:, i:i + 1], 0.0,
                    mybir.AluOpType.add, mybir.AluOpType.max)
                return yb

            TAP_ORDER = (4, 0, 1, 2, 3, 5, 6, 7, 8)

            def st1_pe(i, s):
                r = RATES[i]
                H0 = s * STRIP
                yb = ypool.tile([CQ, SNT], BF16, tag="y")
                for half in range(2):
                    h0 = H0 + half * 2 * RPB
                    p1 = ps1.tile([CQ, 2, BANK], F32, tag="p1")
                    bidx = [0, 0]
                    btot = [len(_tap_list(r, h0 + b * RPB, h0 + (b + 1) * RPB))
                            for b in range(2)]
                    for t in TAP_ORDER:
                        dh, dw = t // 3 - 1, t % 3 - 1
                        for b in range(2):
                            bh0 = h0 + b * RPB
                            bh1 = bh0 + RPB
                            lo = max(bh0, -dh * r)
                            hi = min(bh1, H - dh * r)
                            if lo >= hi:
                                continue
                            rhs = xpad[i][:, lo + dh * r:hi + dh * r,
                                          r + dw * r:r + dw * r + W]
                            j = bidx[b]
                            bidx[b] += 1
                            nc.tensor.matmul(
                                p1[:, b, (lo - bh0) * W:(hi - bh0) * W],
                                dww[:, i * 9 + t, :], rhs,
                                start=(j == 0), stop=(j == btot[b] - 1))
                    yh = yb[:, half * 2 * NT:(half + 1) * 2 * NT]
                    nc.scalar.activation(
                        yh.rearrange("p (b n) -> p b n", b=2), p1[:, :, 0:NT],
                        mybir.ActivationFunctionType.Relu,
                        bias=s1b[:, i:i + 1], scale=1.0)
                return yb

            def st1_strip(s):
                ys = []
                for i in range(4):
                    if ON_PE[(i, s)]:
                        ys.append(st1_pe(i, s))
                    else:
                        ys.append(st1_dve(i, s))
                return ys

            # ---- stage 2 + stage 3 for one strip
            CHUNKS = SNT // BANK  # 3

            def st2_strip(s, ys):
                zs = []
                for o in range(4):
                    z = zpool.tile([CQ, SNT], BF16, tag="z")
                    if ST2_ON_PE[o]:
                        for c in range(CHUNKS):
                            p2 = ps2.tile([CQ, BANK], F32, tag="p2")
                            for i in range(4):
                                nc.tensor.matmul(
                                    p2[:], mixw[:, o * 4 + i, :],
                                    ys[i][:, c * BANK:(c + 1) * BANK],
                                    start=(i == 0), stop=(i == 3))
                            nc.scalar.activation(
                                z[:, c * BANK:(c + 1) * BANK], p2[:],
                                mybir.ActivationFunctionType.Relu,
                                bias=b2bc[:, o:o + 1], scale=1.0)
                    else:
                        acc = a2pool.tile([CQ, SNT], FP16, tag="acc2")
                        nc.vector.tensor_scalar(
                            acc[:], ys[0][:], amix[:, o * 4:o * 4 + 1], None,
                            mybir.AluOpType.mult)
                        for i in range(1, 4):
                            nc.vector.scalar_tensor_tensor(
                                acc[:], ys[i][:],
                                amix[:, o * 4 + i:o * 4 + i + 1], acc[:],
                                mybir.AluOpType.mult, mybir.AluOpType.add)
                        nc.vector.tensor_scalar(
                            z[:], acc[:], b2bc[:, o:o + 1], 0.0,
                            mybir.AluOpType.add, mybir.AluOpType.max)
                    zs.append(z)
                return zs

            def st3_strip(s, zs):
                H0 = s * STRIP
                for m in range(4):
                    ot = opool.tile([CQ, SNT], F32, tag="ot")
                    for c in range(CHUNKS):
                        p3 = ps3.tile([CQ, BANK], F32, tag="p3")
                        for o in range(4):
                            nc.tensor.matmul(
                                p3[:], s3w[:, m * 4 + o, :],
                                zs[o][:, c * BANK:(c + 1) * BANK],
                                start=(o == 0), stop=(o == 3))
                        nc.scalar.activation(
                            ot[:, c * BANK:(c + 1) * BANK], p3[:],
                            mybir.ActivationFunctionType.Relu,
                            bias=b3[:, m:m + 1], scale=1.0)
                    nc.sync.dma_start(
                        out_d[CQ * m:CQ * (m + 1), H0 * W:H0 * W + SNT], ot[:])

            # ---- software pipeline: st1 one strip ahead of st2/st3
            load_strip(0)
            load_strip(1)
            ys_prev = None
            for s in range(N_STRIPS + 1):
                if s < N_STRIPS:
                    if s + 2 < N_STRIPS:
                        load_strip(s + 2)
                    ys_cur = st1_strip(s)
                if s >= 1:
                    zs = st2_strip(s - 1, ys_prev)
                    st3_strip(s - 1, zs)
                ys_prev = ys_cur

          if loop_n:
              with tc.For_i(0, loop_n, 1):
                  _body()
          else:
              _body()
    _split_multi_waits(nc)
    return nc


def _get_program(loop_n=None):
    key = ("nc", loop_n)
    if key not in _PROG_CACHE:
        _PROG_CACHE[key] = _build_program(loop_n)
    return _PROG_CACHE[key]


def _in_maps(x, consts):
    x = np.ascontiguousarray(np.asarray(x, np.float32))
    maps = []
    for b in range(B):
        m = dict(consts)
        m["x"] = np.ascontiguousarray(x[b].reshape(C, H, W))
        maps.append(m)
    return maps


def run(x, consts, trace=False, loop_n=None):
    nc = _get_program(loop_n)
    res = run_bass_kernel_spmd(nc, _in_maps(x, consts), list(range(B)),
                               trace=trace)
    out = np.stack([res.results[b]["out"].reshape(C, H, W) for b in range(B)])
    return out.astype(np.float32), res


def kernel(x, wdw, bdw, gdw, bedw, mdw, vdw, ws, bs, gs, bes, ms, vs,
           wo, bo, go, beo, mo, vo):
    consts = _host_consts(wdw, bdw, gdw, bedw, mdw, vdw, ws, bs, gs, bes,
                          ms, vs, wo, bo, go, beo, mo, vo)
    out, _ = run(x, consts, trace=False)
    return out
